# revision 1
# baseline (speedup 1.0000x reference)
"""BitFlipLayer Trainium2 kernel.

out = bitcast_f32( bits(x) ^ ((flip_mask << 31) >>logical bit_pos) )

Pure elementwise bit manipulation; memory-bound (16 B/elem HBM traffic).
Sharded data-parallel over 8 NeuronCores along the leading dim.
All on-chip compute in uint32: one fused scalar_tensor_tensor (DVE) builds
the single-bit flip word, one tensor_tensor xor applies it.
"""

import numpy as np

import concourse.bacc as bacc
import concourse.mybir as mybir
from concourse.mybir import AluOpType
from concourse.tile import TileContext
from concourse.bass_utils import run_bass_kernel_spmd

N_CORES = 8
FULL_SHAPE = (32, 1024, 1024)
ELEMS_PER_CORE = (FULL_SHAPE[0] // N_CORES) * FULL_SHAPE[1] * FULL_SHAPE[2]  # 4 Mi
P = 128
FD = 2048  # free-dim elems per tile -> [128, 2048] u32 = 1 MiB per DMA
NT = ELEMS_PER_CORE // (P * FD)  # 16 tiles


def _stt_imm(eng, out, in0, imm, in1, op0, op1):
    """out = (in0 op0 imm) op1 in1 with an integer-typed immediate.

    The python scalar_tensor_tensor builder hardcodes a float32 immediate,
    which the walrus verifier rejects for bitvec ops ("ImmVal must be
    integer and match the type of src and dst").
    """
    ins_obj = mybir.InstTensorScalarPtr(
        name=eng.bass.get_next_instruction_name(),
        is_scalar_tensor_tensor=True,
        op0=op0,
        op1=op1,
        ins=[
            eng.lower_ap(in0),
            mybir.ImmediateValue(dtype=out.dtype, value=imm),
            eng.lower_ap(in1),
        ],
        outs=[eng.lower_ap(out)],
    )
    return eng.add_instruction(ins_obj)


_NC_CACHE = None


def _build():
    global _NC_CACHE
    if _NC_CACHE is not None:
        return _NC_CACHE

    u32 = mybir.dt.uint32
    nc = bacc.Bacc(None, target_bir_lowering=False)
    x = nc.dram_tensor("x", [ELEMS_PER_CORE], u32, kind="ExternalInput")
    bp = nc.dram_tensor("bit_pos", [ELEMS_PER_CORE], u32, kind="ExternalInput")
    fm = nc.dram_tensor("flip_mask", [ELEMS_PER_CORE], u32, kind="ExternalInput")
    out = nc.dram_tensor("out", [ELEMS_PER_CORE], u32, kind="ExternalOutput")

    xv = x[:].rearrange("(n p f) -> n p f", p=P, f=FD)
    bpv = bp[:].rearrange("(n p f) -> n p f", p=P, f=FD)
    fmv = fm[:].rearrange("(n p f) -> n p f", p=P, f=FD)
    outv = out[:].rearrange("(n p f) -> n p f", p=P, f=FD)

    with TileContext(nc) as tc:
        with (
            tc.tile_pool(name="xp", bufs=4) as xp,
            tc.tile_pool(name="bpp", bufs=4) as bpp,
            tc.tile_pool(name="fmp", bufs=4) as fmp,
        ):
            for i in range(NT):
                xt = xp.tile([P, FD], u32)
                bt = bpp.tile([P, FD], u32)
                ft = fmp.tile([P, FD], u32)
                # loads on the SP HWDGE ring
                nc.sync.dma_start(xt[:], xv[i, :, :])
                nc.sync.dma_start(bt[:], bpv[i, :, :])
                nc.sync.dma_start(ft[:], fmv[i, :, :])
                # ft = (ft << 31) >>L bt   (single-bit flip word, or 0)
                _stt_imm(
                    nc.vector, ft[:], ft[:], 31, bt[:],
                    AluOpType.logical_shift_left,
                    AluOpType.logical_shift_right,
                )
                # xt ^= ft
                nc.vector.tensor_tensor(xt[:], xt[:], ft[:], op=AluOpType.bitwise_xor)
                # store on the ACT HWDGE ring
                nc.scalar.dma_start(outv[i, :, :], xt[:])

    nc.finalize()
    _NC_CACHE = nc
    return nc


def kernel(x: np.ndarray, bit_pos: np.ndarray, flip_mask: np.ndarray) -> np.ndarray:
    nc = _build()

    planes = FULL_SHAPE[0] // N_CORES
    xs = np.ascontiguousarray(x).view(np.uint32).reshape(N_CORES, ELEMS_PER_CORE)
    bps = np.ascontiguousarray(bit_pos).view(np.uint32).reshape(N_CORES, ELEMS_PER_CORE)
    fms = np.ascontiguousarray(flip_mask).view(np.uint32).reshape(N_CORES, ELEMS_PER_CORE)

    in_maps = [
        {"x": xs[c], "bit_pos": bps[c], "flip_mask": fms[c]}
        for c in range(N_CORES)
    ]
    res = run_bass_kernel_spmd(nc, in_maps, core_ids=list(range(N_CORES)))

    out = np.empty(FULL_SHAPE, dtype=np.float32)
    of = out.reshape(N_CORES, ELEMS_PER_CORE)
    for c in range(N_CORES):
        of[c] = res.results[c]["out"].view(np.float32)
    return out


# revision 6
# speedup vs baseline: 17.0869x; 17.0869x over previous
"""BitFlipLayer Trainium2 kernel.

out = bitcast_f32( bits(x) ^ ((flip_mask << 31) >>logical bit_pos) )

Pure elementwise bit manipulation; memory-bound (16 B/elem HBM traffic).
Sharded data-parallel over 8 NeuronCores along the leading dim.
All on-chip compute in uint32: one fused scalar_tensor_tensor (DVE) builds
the single-bit flip word, one tensor_tensor xor applies it.
"""

import numpy as np

import concourse.bacc as bacc
import concourse.mybir as mybir
from concourse.mybir import AluOpType
from concourse.tile import TileContext
from concourse.bass_utils import run_bass_kernel_spmd

N_CORES = 8
FULL_SHAPE = (32, 1024, 1024)
ELEMS_PER_CORE = (FULL_SHAPE[0] // N_CORES) * FULL_SHAPE[1] * FULL_SHAPE[2]  # 4 Mi
P = 128
FD = 2048  # free-dim elems per tile -> [128, 2048] u32 = 1 MiB per DMA
NT = ELEMS_PER_CORE // (P * FD)  # 16 tiles


def _stt_imm(eng, out, in0, imm, in1, op0, op1):
    """out = (in0 op0 imm) op1 in1 with an integer-typed immediate.

    The python scalar_tensor_tensor builder hardcodes a float32 immediate,
    which the walrus verifier rejects for bitvec ops ("ImmVal must be
    integer and match the type of src and dst").
    """
    ins_obj = mybir.InstTensorScalarPtr(
        name=eng.bass.get_next_instruction_name(),
        is_scalar_tensor_tensor=True,
        op0=op0,
        op1=op1,
        ins=[
            eng.lower_ap(in0),
            mybir.ImmediateValue(dtype=out.dtype, value=imm),
            eng.lower_ap(in1),
        ],
        outs=[eng.lower_ap(out)],
    )
    return eng.add_instruction(ins_obj)


_NC_CACHE = {}


def _build(nt=None, loops=1, fd=FD, bufs=4, split=False):
    if nt is None:
        nt = ELEMS_PER_CORE // (P * fd)
    key = (nt, loops, fd, bufs, split)
    if key in _NC_CACHE:
        return _NC_CACHE[key]

    u32 = mybir.dt.uint32
    nc = bacc.Bacc(None, target_bir_lowering=False)
    x = nc.dram_tensor("x", [ELEMS_PER_CORE], u32, kind="ExternalInput")
    bp = nc.dram_tensor("bit_pos", [ELEMS_PER_CORE], u32, kind="ExternalInput")
    fm = nc.dram_tensor("flip_mask", [ELEMS_PER_CORE], u32, kind="ExternalInput")
    out = nc.dram_tensor("out", [ELEMS_PER_CORE], u32, kind="ExternalOutput")

    xv = x[:].rearrange("(n p f) -> n p f", p=P, f=fd)
    bpv = bp[:].rearrange("(n p f) -> n p f", p=P, f=fd)
    fmv = fm[:].rearrange("(n p f) -> n p f", p=P, f=fd)
    outv = out[:].rearrange("(n p f) -> n p f", p=P, f=fd)

    with TileContext(nc) as tc:
        import contextlib
        loop_cm = tc.For_i(0, loops, 1) if loops > 1 else contextlib.nullcontext()
        with (
            loop_cm,
            tc.tile_pool(name="xp", bufs=bufs) as xp,
            tc.tile_pool(name="bpp", bufs=bufs) as bpp,
            tc.tile_pool(name="fmp", bufs=bufs) as fmp,
        ):
            for i in range(nt):
                xt = xp.tile([P, fd], u32)
                bt = bpp.tile([P, fd], u32)
                ft = fmp.tile([P, fd], u32)
                # loads on the HWDGE rings
                nc.sync.dma_start(xt[:], xv[i, :, :])
                (nc.scalar if split else nc.sync).dma_start(bt[:], bpv[i, :, :])
                nc.sync.dma_start(ft[:], fmv[i, :, :])
                # ft = (ft << 31) >>L bt   (single-bit flip word, or 0)
                _stt_imm(
                    nc.vector, ft[:], ft[:], 31, bt[:],
                    AluOpType.logical_shift_left,
                    AluOpType.logical_shift_right,
                )
                # xt ^= ft
                nc.vector.tensor_tensor(xt[:], xt[:], ft[:], op=AluOpType.bitwise_xor)
                # store on the ACT HWDGE ring
                nc.scalar.dma_start(outv[i, :, :], xt[:])

    nc.finalize()
    _NC_CACHE[key] = nc
    return nc


def kernel(x: np.ndarray, bit_pos: np.ndarray, flip_mask: np.ndarray) -> np.ndarray:
    nc = _build()

    def u32_shards(a):
        a = np.ascontiguousarray(np.asarray(a))
        return a.view(np.uint32).reshape(N_CORES, ELEMS_PER_CORE)

    xs = u32_shards(x)
    bps = u32_shards(bit_pos)
    fms = u32_shards(flip_mask)

    in_maps = [
        {"x": xs[c], "bit_pos": bps[c], "flip_mask": fms[c]}
        for c in range(N_CORES)
    ]
    res = run_bass_kernel_spmd(nc, in_maps, core_ids=list(range(N_CORES)))

    out = np.empty(FULL_SHAPE, dtype=np.float32)
    of = out.reshape(N_CORES, ELEMS_PER_CORE)
    for c in range(N_CORES):
        of[c] = res.results[c]["out"].view(np.float32)
    return out



# revision 7
# speedup vs baseline: 17.1914x; 1.0061x over previous
"""BitFlipLayer Trainium2 kernel.

out = bitcast_f32( bits(x) ^ ((flip_mask << 31) >>logical bit_pos) )

Pure elementwise bit manipulation; memory-bound (16 B/elem HBM traffic).
Sharded data-parallel over 8 NeuronCores along the leading dim.
All on-chip compute in uint32: one fused scalar_tensor_tensor (DVE) builds
the single-bit flip word, one tensor_tensor xor applies it.
"""

import numpy as np

import concourse.bacc as bacc
import concourse.mybir as mybir
from concourse.mybir import AluOpType
from concourse.tile import TileContext
from concourse.bass_utils import run_bass_kernel_spmd

N_CORES = 8
FULL_SHAPE = (32, 1024, 1024)
ELEMS_PER_CORE = (FULL_SHAPE[0] // N_CORES) * FULL_SHAPE[1] * FULL_SHAPE[2]  # 4 Mi
P = 128
FD = 2048  # free-dim elems per tile -> [128, 2048] u32 = 1 MiB per DMA
NT = ELEMS_PER_CORE // (P * FD)  # 16 tiles


def _stt_imm(eng, out, in0, imm, in1, op0, op1):
    """out = (in0 op0 imm) op1 in1 with an integer-typed immediate.

    The python scalar_tensor_tensor builder hardcodes a float32 immediate,
    which the walrus verifier rejects for bitvec ops ("ImmVal must be
    integer and match the type of src and dst").
    """
    ins_obj = mybir.InstTensorScalarPtr(
        name=eng.bass.get_next_instruction_name(),
        is_scalar_tensor_tensor=True,
        op0=op0,
        op1=op1,
        ins=[
            eng.lower_ap(in0),
            mybir.ImmediateValue(dtype=out.dtype, value=imm),
            eng.lower_ap(in1),
        ],
        outs=[eng.lower_ap(out)],
    )
    return eng.add_instruction(ins_obj)


_NC_CACHE = {}


def _build(nt=None, loops=1, fd=FD, bufs=4, split=False, staggered=False):
    if nt is None:
        nt = ELEMS_PER_CORE // (P * fd)
    key = (nt, loops, fd, bufs, split, staggered)
    if key in _NC_CACHE:
        return _NC_CACHE[key]

    u32 = mybir.dt.uint32
    nc = bacc.Bacc(None, target_bir_lowering=False)
    x = nc.dram_tensor("x", [ELEMS_PER_CORE], u32, kind="ExternalInput")
    bp = nc.dram_tensor("bit_pos", [ELEMS_PER_CORE], u32, kind="ExternalInput")
    fm = nc.dram_tensor("flip_mask", [ELEMS_PER_CORE], u32, kind="ExternalInput")
    out = nc.dram_tensor("out", [ELEMS_PER_CORE], u32, kind="ExternalOutput")

    xv = x[:].rearrange("(n p f) -> n p f", p=P, f=fd)
    bpv = bp[:].rearrange("(n p f) -> n p f", p=P, f=fd)
    fmv = fm[:].rearrange("(n p f) -> n p f", p=P, f=fd)
    outv = out[:].rearrange("(n p f) -> n p f", p=P, f=fd)

    with TileContext(nc) as tc:
        import contextlib
        loop_cm = tc.For_i(0, loops, 1, staggered_reset=staggered) if loops > 1 else contextlib.nullcontext()
        with (
            loop_cm,
            tc.tile_pool(name="xp", bufs=bufs) as xp,
            tc.tile_pool(name="bpp", bufs=bufs) as bpp,
            tc.tile_pool(name="fmp", bufs=bufs) as fmp,
        ):
            for i in range(nt):
                xt = xp.tile([P, fd], u32)
                bt = bpp.tile([P, fd], u32)
                ft = fmp.tile([P, fd], u32)
                # loads on the HWDGE rings
                nc.sync.dma_start(xt[:], xv[i, :, :])
                (nc.scalar if split else nc.sync).dma_start(bt[:], bpv[i, :, :])
                nc.sync.dma_start(ft[:], fmv[i, :, :])
                # ft = (ft << 31) >>L bt   (single-bit flip word, or 0)
                _stt_imm(
                    nc.vector, ft[:], ft[:], 31, bt[:],
                    AluOpType.logical_shift_left,
                    AluOpType.logical_shift_right,
                )
                # xt ^= ft
                nc.vector.tensor_tensor(xt[:], xt[:], ft[:], op=AluOpType.bitwise_xor)
                # store on the ACT HWDGE ring
                nc.scalar.dma_start(outv[i, :, :], xt[:])

    nc.finalize()
    _NC_CACHE[key] = nc
    return nc


def kernel(x: np.ndarray, bit_pos: np.ndarray, flip_mask: np.ndarray) -> np.ndarray:
    nc = _build()

    def u32_shards(a):
        a = np.ascontiguousarray(np.asarray(a))
        return a.view(np.uint32).reshape(N_CORES, ELEMS_PER_CORE)

    xs = u32_shards(x)
    bps = u32_shards(bit_pos)
    fms = u32_shards(flip_mask)

    in_maps = [
        {"x": xs[c], "bit_pos": bps[c], "flip_mask": fms[c]}
        for c in range(N_CORES)
    ]
    res = run_bass_kernel_spmd(nc, in_maps, core_ids=list(range(N_CORES)))

    out = np.empty(FULL_SHAPE, dtype=np.float32)
    of = out.reshape(N_CORES, ELEMS_PER_CORE)
    for c in range(N_CORES):
        of[c] = res.results[c]["out"].view(np.float32)
    return out

